# revision 56
# baseline (speedup 1.0000x reference)
"""Trainium2 Bass kernel for LoopABMIL (attention-based MIL pooling).

reference:
    h = silu(x @ Wp + bp)            # [B, N, H]
    a = h @ Wa[:, 0] + ba            # [B, N]
    p = softmax(a masked to lengths) # [B, N]
    pooled = p @ h                   # [B, H]
    logits = pooled @ Wc + bc        # [B, C]

Sharding: softmax-pooling is associative, so each of the 8 cores processes an
equal number of 128-patch chunks from EVERY bag (round-robin over the bag's
valid chunks only — patches beyond lengths[b] are never loaded or computed).
Each core emits per-bag partials (sum_p e^{a_p} h_p and sum_p e^{a_p}); the
host merges partials across cores and applies the tiny classifier.  ba
cancels in the softmax ratio and is dropped on device.

Two-pass device structure (ScalarE LUT table sets: Silu and Exp live in
different sets and a set switch costs ~2.7us, so never interleave them):
  pass 1 (per chunk): DMA x superchunk -> 8 accumulating matmuls + 1 bias
      matmul -> PSUM -> Silu (ScalarE, one table set) -> h stash in SBUF
      (bf16) -> VectorE mul+reduce for attention logit column a_all[:, g].
  pass 2 (per bag): VectorE adds the ragged mask (0 / -30000) to the bag's
      a_all columns, one batched Exp (ScalarE) -> w_all columns, then
      accumulating [1x128 @ 128x256] pooled matmuls over the bag's chunks.
      One final [128xG @ 128x1] matmul yields all per-chunk weight sums.
"""

import sys

if "/opt/trn_rl_repo" not in sys.path:
    sys.path.insert(0, "/opt/trn_rl_repo")

from contextlib import ExitStack

import ml_dtypes
import numpy as np

import concourse.bacc as bacc
import concourse.tile as tile
from concourse import mybir
from concourse.bass_utils import run_bass_kernel_spmd

B, N, D, H, C = 8, 8192, 1024, 256, 2
P = 128          # patch chunk size (SBUF partitions)
NCORES = 8
KT = D // P      # k-tiles in the projection contraction
NEG = -30000.0   # additive mask: exp(a + NEG) == 0.0 exactly in f32
S = 8            # max chunks per DMA superchunk (2 MiB per transfer)


def _dma_schedule(G: int):
    """Superchunk sizes: small first transfers so the PE starts early."""
    sizes = []
    left = G
    for want in (1, 2, 4, 6):
        if left <= 0:
            break
        sizes.append(min(want, left))
        left -= sizes[-1]
    while left > 0:
        sizes.append(min(S, left))
        left -= sizes[-1]
    return sizes

BF = mybir.dt.bfloat16
F32 = mybir.dt.float32

_cache: dict = {}


def _build(G: int, n_per_bag: tuple, act=None) -> "bacc.Bacc":
    """One SPMD program shared by all 8 cores: G chunks grouped by bag."""
    if act is None:
        act = mybir.ActivationFunctionType.Silu
    nc = bacc.Bacc("TRN2", target_bir_lowering=False)

    # const blob layout (bf16 columns, partition-major so one DMA loads all):
    #   [0 : KT*H)          wp k-tiles
    #   [KT*H : KT*H+4*H)   wab replicated x4
    #   [KT*H+4*H : +G)     maskT as bf16? no - mask needs f32; separate tensor
    #   bp lives in partition 0 of its own column range [.. +H)
    CW = KT * H + 4 * H + H
    xpk = nc.dram_tensor("xpk", [P, G * D], BF, kind="ExternalInput")
    cblob = nc.dram_tensor("cblob", [P, CW], BF, kind="ExternalInput")
    maskT = nc.dram_tensor("maskT", [P, G], F32, kind="ExternalInput")
    out = nc.dram_tensor("out", [1, B * H + G], F32, kind="ExternalOutput")

    with tile.TileContext(nc) as tc, ExitStack() as ctx:
        const = ctx.enter_context(tc.tile_pool(name="const", bufs=1))
        xpool = ctx.enter_context(tc.tile_pool(name="xp", bufs=4))
        hprep = ctx.enter_context(tc.tile_pool(name="hpre", bufs=3, space="PSUM"))
        store = ctx.enter_context(tc.tile_pool(name="store", bufs=1))
        scrp = ctx.enter_context(tc.tile_pool(name="scr", bufs=4))
        smallp = ctx.enter_context(tc.tile_pool(name="small", bufs=4))
        poolp = ctx.enter_context(tc.tile_pool(name="poolps", bufs=2, space="PSUM"))
        sps = ctx.enter_context(tc.tile_pool(name="sps", bufs=1, space="PSUM"))
        outp = ctx.enter_context(tc.tile_pool(name="outp", bufs=1))

        Q = 4  # chunks fused per PSUM tile (4*H f32 = 2 banks) / ACT / DVE op
        # HAM warm-up: junk matmuls gated only on a memset, sized to finish
        # about when the first x data lands (~11us), so the real stream
        # starts at 2.4 GHz with no idle gap.
        warm_in = const.tile([P, P], BF, tag="warmin")
        nc.vector.memset(warm_in, 0.0)
        warmp = ctx.enter_context(tc.tile_pool(name="warm", bufs=1, space="PSUM"))
        warmp2 = ctx.enter_context(tc.tile_pool(name="warm2", bufs=1, space="PSUM"))
        wps = warmp.tile([P, P], F32, tag="warmps")
        NWARM = 44
        for i in range(NWARM):
            nc.tensor.matmul(
                wps, lhsT=warm_in, rhs=warm_in,
                start=(i == 0), stop=(i == NWARM - 1),
            )
        cb = const.tile([P, CW], BF, tag="cblob")
        nc.sync.dma_start(out=cb, in_=cblob[:])
        wp_t = [cb[:, k * H:(k + 1) * H] for k in range(KT)]
        wab_t = cb[:, KT * H:KT * H + Q * H]
        bp_t = cb[0:1, KT * H + Q * H:KT * H + Q * H + H]
        mask_t = const.tile([P, G], F32, tag="mask")
        nc.gpsimd.dma_start(out=mask_t, in_=maskT[:])
        ones_w = const.tile([1, P], BF, tag="onesw")   # bias-broadcast lhsT
        nc.vector.memset(ones_w, 1.0)
        ones_n = const.tile([P, 1], BF, tag="onesn")   # denominator rhs
        nc.vector.memset(ones_n, 1.0)



        hstore = store.tile([P, G * H], BF, tag="hstore")
        a_all = store.tile([P, G], F32, tag="a_all")
        w_all = store.tile([P, G], BF, tag="w_all")
        out_sb = outp.tile([1, B * H], F32, tag="outsb")

        # ---- pass 1: projection + silu + attention logits, all chunks ----
        sched = _dma_schedule(G)
        sup_start = {}
        g0 = 0
        for ns in sched:
            sup_start[g0] = ns
            g0 += ns
        xsup = None
        sup_base = 0
        for g in range(G):
            if g in sup_start:
                ns = sup_start[g]
                sup_base = g
                xsup = xpool.tile([P, S * D], BF, tag="xt")
                nc.sync.dma_start(
                    out=xsup[:, 0:ns * D],
                    in_=xpk[:, g * D:(g + ns) * D],
                )
            xt = xsup[:, (g - sup_base) * D:(g - sup_base + 1) * D]
            hp = hprep.tile([P, H], F32, tag="hp")
            nc.tensor.matmul(hp, lhsT=ones_w, rhs=bp_t, start=True, stop=False)
            for k in range(KT):
                nc.tensor.matmul(
                    hp,
                    lhsT=xt[:, k * P:(k + 1) * P],
                    rhs=wp_t[k],
                    start=False,
                    stop=(k == KT - 1),
                )

            hout = hstore[:, g * H:(g + 1) * H]
            nc.scalar.activation(out=hout, in_=hp, func=act)

            scr = scrp.tile([P, H], BF, tag="scr")
            # split the elementwise mul between GpSimd and DVE; the final
            # chunks go to DVE (faster) to shorten the pass-2 gating chain
            mul_eng = nc.vector if g % 2 == 1 or g + 4 >= G else nc.gpsimd
            mul_eng.tensor_mul(scr, hout, wab_t[:, 0:H])
            nc.vector.reduce_sum(
                out=a_all[:, g:g + 1], in_=scr, axis=mybir.AxisListType.X
            )

        # bridge the pass-1 -> pass-2 PE idle (exp chain wait) with junk
        # matmuls in their own PSUM bank so the pooled burst stays at 2.4 GHz
        wps2 = warmp2.tile([P, P], F32, tag="warmps2")
        for i in range(16):
            nc.tensor.matmul(
                wps2, lhsT=warm_in, rhs=warm_in,
                start=(i == 0), stop=(i == 15),
            )

        # ---- pass 2: one global masked exp (avoids ACT table-set thrash
        # between Silu and Exp), then per-bag pooled accumulation ----
        am = store.tile([P, G], F32, tag="am")
        nc.vector.tensor_add(am, a_all, mask_t)
        nc.scalar.activation(
            out=w_all, in_=am, func=mybir.ActivationFunctionType.Exp
        )
        off = 0
        for b in range(B):
            nb = n_per_bag[b]
            pool_t = poolp.tile([1, H], F32, tag="pool")
            for j in range(nb):
                g = off + j
                nc.tensor.matmul(
                    pool_t,
                    lhsT=w_all[:, g:g + 1],
                    rhs=hstore[:, g * H:(g + 1) * H],
                    start=(j == 0),
                    stop=(j == nb - 1),
                )
            nc.vector.tensor_copy(out_sb[0:1, b * H:(b + 1) * H], pool_t)
            nc.sync.dma_start(
                out=out[0:1, b * H:(b + 1) * H],
                in_=out_sb[0:1, b * H:(b + 1) * H],
            )
            off += nb

        s_ps = sps.tile([G, 1], F32, tag="sps")
        nc.tensor.matmul(s_ps, lhsT=w_all, rhs=ones_n, start=True, stop=True)
        s_sb = smallp.tile([G, 1], F32, tag="ssb")
        nc.vector.tensor_copy(s_sb, s_ps)
        nc.sync.dma_start(out=out[0:1, B * H:B * H + G], in_=s_sb)

    nc.compile()
    return nc


def _plan(lengths: np.ndarray):
    """Chunk counts: bag b has T_b valid chunks; every core gets n_b slots."""
    lens = np.asarray(lengths, dtype=np.int64)
    T = np.maximum((lens + P - 1) // P, 1)       # valid chunks per bag
    n = (T + NCORES - 1) // NCORES               # per-core slots per bag
    G = int(n.sum())
    return T, n, G


def _pack(x, lengths, T, n, G):
    """Per-core inputs: xpk [G,128,1024] bf16 (lhsT layout) + maskT [128,G]."""
    lens = np.asarray(lengths, dtype=np.int64)
    # x[b, t*128+p, k*128+d] -> xr[b, t, d, k*128+p]  (d = within-k-tile index)
    xr = (
        np.asarray(x)
        .astype(ml_dtypes.bfloat16)
        .reshape(B, N // P, P, KT, P)
        .transpose(0, 1, 4, 3, 2)
        .reshape(B, N // P, P, D)
    )
    bs = np.repeat(np.arange(B), n)
    in_maps = []
    for c in range(NCORES):
        js = np.concatenate([np.arange(nb) for nb in n])
        ts = c + NCORES * js                       # global chunk id per slot
        ts_clip = np.minimum(ts, T[bs] - 1)
        xpk = xr[bs, ts_clip]                      # [G, 128, 1024] bf16
        # partition-major: one contiguous G*2KB run per partition
        xpk_t = np.ascontiguousarray(xpk.transpose(1, 0, 2)).reshape(P, G * D)
        # valid patches in slot: clip(len - t*128, 0, 128); dummies get 0
        valid = np.clip(lens[bs] - ts * P, 0, P)
        valid[ts >= T[bs]] = 0
        maskT = np.where(
            np.arange(P)[:, None] < valid[None, :], 0.0, NEG
        ).astype(np.float32)
        in_maps.append({"xpk": xpk_t, "maskT": maskT})
    return in_maps, bs


def _run(inputs: dict, trace: bool = False):
    x = np.asarray(inputs["x"], dtype=np.float32)
    lengths = np.asarray(inputs["lengths"])
    Wp = np.asarray(inputs["Wp"], dtype=np.float32)
    bp = np.asarray(inputs["bp"], dtype=np.float32)
    Wa = np.asarray(inputs["Wa"], dtype=np.float32)
    Wc = np.asarray(inputs["Wc"], dtype=np.float32)
    bc = np.asarray(inputs["bc"], dtype=np.float32)

    T, n, G = _plan(lengths)
    key = (G, tuple(int(v) for v in n))
    if key not in _cache:
        _cache[key] = _build(G, key[1])
    nc = _cache[key]

    in_maps, bs = _pack(x, lengths, T, n, G)
    CW = KT * H + 4 * H + H
    cblob = np.zeros((P, CW), dtype=ml_dtypes.bfloat16)
    # wp k-tiles: cblob[d_in, k*H + h] = Wp[k*P + d_in, h]
    cblob[:, 0:KT * H] = (
        Wp.reshape(KT, P, H).transpose(1, 0, 2).reshape(P, KT * H)
        .astype(ml_dtypes.bfloat16)
    )
    cblob[:, KT * H:KT * H + 4 * H] = np.tile(
        Wa[:, 0][None, :], (P, 4)
    ).astype(ml_dtypes.bfloat16)
    cblob[0, KT * H + 4 * H:CW] = bp.astype(ml_dtypes.bfloat16)
    for m in in_maps:
        m["cblob"] = cblob

    res = run_bass_kernel_spmd(
        nc, in_maps, core_ids=list(range(NCORES)), trace=trace
    )

    v = np.zeros((B, H), np.float64)
    s = np.zeros(B, np.float64)
    for r in res.results:
        flat = r["out"].reshape(-1).astype(np.float64)
        v += flat[:B * H].reshape(B, H)
        np.add.at(s, bs, flat[B * H:B * H + G])
    pooled = v / s[:, None]
    logits = pooled @ Wc.astype(np.float64) + bc.astype(np.float64)
    return logits.astype(np.float32), res.exec_time_ns


def kernel(**inputs) -> np.ndarray:
    logits, _ = _run(inputs, trace=False)
    return logits


# revision 57
# speedup vs baseline: 1.0044x; 1.0044x over previous
"""Trainium2 Bass kernel for LoopABMIL (attention-based MIL pooling).

reference:
    h = silu(x @ Wp + bp)            # [B, N, H]
    a = h @ Wa[:, 0] + ba            # [B, N]
    p = softmax(a masked to lengths) # [B, N]
    pooled = p @ h                   # [B, H]
    logits = pooled @ Wc + bc        # [B, C]

Sharding: softmax-pooling is associative, so each of the 8 cores processes an
equal number of 128-patch chunks from EVERY bag (round-robin over the bag's
valid chunks only — patches beyond lengths[b] are never loaded or computed).
Each core emits per-bag partials (sum_p e^{a_p} h_p and sum_p e^{a_p}); the
host merges partials across cores and applies the tiny classifier.  ba
cancels in the softmax ratio and is dropped on device.

Two-pass device structure (ScalarE LUT table sets: Silu and Exp live in
different sets and a set switch costs ~2.7us, so never interleave them):
  pass 1 (per chunk): DMA x superchunk -> 8 accumulating matmuls + 1 bias
      matmul -> PSUM -> Silu (ScalarE, one table set) -> h stash in SBUF
      (bf16) -> VectorE mul+reduce for attention logit column a_all[:, g].
  pass 2 (per bag): VectorE adds the ragged mask (0 / -30000) to the bag's
      a_all columns, one batched Exp (ScalarE) -> w_all columns, then
      accumulating [1x128 @ 128x256] pooled matmuls over the bag's chunks.
      One final [128xG @ 128x1] matmul yields all per-chunk weight sums.
"""

import sys

if "/opt/trn_rl_repo" not in sys.path:
    sys.path.insert(0, "/opt/trn_rl_repo")

from contextlib import ExitStack

import ml_dtypes
import numpy as np

import concourse.bacc as bacc
import concourse.tile as tile
from concourse import mybir
from concourse.bass_utils import run_bass_kernel_spmd

B, N, D, H, C = 8, 8192, 1024, 256, 2
P = 128          # patch chunk size (SBUF partitions)
NCORES = 8
KT = D // P      # k-tiles in the projection contraction
NEG = -30000.0   # additive mask: exp(a + NEG) == 0.0 exactly in f32
S = 8            # max chunks per DMA superchunk (2 MiB per transfer)


def _dma_schedule(G: int):
    """Superchunk sizes: small first transfers so the PE starts early."""
    sizes = []
    left = G
    for want in (1, 2, 4, 6):
        if left <= 0:
            break
        sizes.append(min(want, left))
        left -= sizes[-1]
    while left > 0:
        sizes.append(min(S, left))
        left -= sizes[-1]
    return sizes

BF = mybir.dt.bfloat16
F32 = mybir.dt.float32

_cache: dict = {}


def _build(G: int, n_per_bag: tuple, act=None) -> "bacc.Bacc":
    """One SPMD program shared by all 8 cores: G chunks grouped by bag."""
    if act is None:
        act = mybir.ActivationFunctionType.Silu
    nc = bacc.Bacc("TRN2", target_bir_lowering=False)

    # const blob layout (bf16 columns, partition-major so one DMA loads all):
    #   [0 : KT*H)          wp k-tiles
    #   [KT*H : KT*H+4*H)   wab replicated x4
    #   [KT*H+4*H : +G)     maskT as bf16? no - mask needs f32; separate tensor
    #   bp lives in partition 0 of its own column range [.. +H)
    CW = KT * H + 4 * H + H
    xpk = nc.dram_tensor("xpk", [P, G * D], BF, kind="ExternalInput")
    cblob = nc.dram_tensor("cblob", [P, CW], BF, kind="ExternalInput")
    maskT = nc.dram_tensor("maskT", [P, G], F32, kind="ExternalInput")
    out = nc.dram_tensor("out", [1, B * H + G], F32, kind="ExternalOutput")

    with tile.TileContext(nc) as tc, ExitStack() as ctx:
        const = ctx.enter_context(tc.tile_pool(name="const", bufs=1))
        xpool = ctx.enter_context(tc.tile_pool(name="xp", bufs=4))
        hprep = ctx.enter_context(tc.tile_pool(name="hpre", bufs=4, space="PSUM"))
        store = ctx.enter_context(tc.tile_pool(name="store", bufs=1))
        scrp = ctx.enter_context(tc.tile_pool(name="scr", bufs=4))
        smallp = ctx.enter_context(tc.tile_pool(name="small", bufs=4))
        poolp = ctx.enter_context(tc.tile_pool(name="poolps", bufs=2, space="PSUM"))
        sps = ctx.enter_context(tc.tile_pool(name="sps", bufs=1, space="PSUM"))
        outp = ctx.enter_context(tc.tile_pool(name="outp", bufs=1))

        Q = 4  # chunks fused per PSUM tile (4*H f32 = 2 banks) / ACT / DVE op
        # HAM warm-up: junk matmuls gated only on a memset, sized to finish
        # about when the first x data lands (~11us), so the real stream
        # starts at 2.4 GHz with no idle gap.
        warm_in = const.tile([P, P], BF, tag="warmin")
        nc.vector.memset(warm_in, 0.0)
        warmp = ctx.enter_context(tc.tile_pool(name="warm", bufs=1, space="PSUM"))
        wps = warmp.tile([P, P], F32, tag="warmps")
        NWARM = 36
        for i in range(NWARM):
            nc.tensor.matmul(
                wps, lhsT=warm_in, rhs=warm_in,
                start=(i == 0), stop=(i == NWARM - 1),
            )
        cb = const.tile([P, CW], BF, tag="cblob")
        nc.sync.dma_start(out=cb, in_=cblob[:])
        wp_t = [cb[:, k * H:(k + 1) * H] for k in range(KT)]
        wab_t = cb[:, KT * H:KT * H + Q * H]
        bp_t = cb[0:1, KT * H + Q * H:KT * H + Q * H + H]
        mask_t = const.tile([P, G], F32, tag="mask")
        nc.gpsimd.dma_start(out=mask_t, in_=maskT[:])
        ones_w = const.tile([1, P], BF, tag="onesw")   # bias-broadcast lhsT
        nc.vector.memset(ones_w, 1.0)
        ones_n = const.tile([P, 1], BF, tag="onesn")   # denominator rhs
        nc.vector.memset(ones_n, 1.0)



        hstore = store.tile([P, G * H], BF, tag="hstore")
        a_all = store.tile([P, G], F32, tag="a_all")
        w_all = store.tile([P, G], BF, tag="w_all")
        out_sb = outp.tile([1, B * H], F32, tag="outsb")

        # ---- pass 1: projection + silu + attention logits, all chunks ----
        sched = _dma_schedule(G)
        sup_start = {}
        g0 = 0
        for ns in sched:
            sup_start[g0] = ns
            g0 += ns
        xsup = None
        sup_base = 0
        for g in range(G):
            if g in sup_start:
                ns = sup_start[g]
                sup_base = g
                xsup = xpool.tile([P, S * D], BF, tag="xt")
                nc.sync.dma_start(
                    out=xsup[:, 0:ns * D],
                    in_=xpk[:, g * D:(g + ns) * D],
                )
            xt = xsup[:, (g - sup_base) * D:(g - sup_base + 1) * D]
            hp = hprep.tile([P, H], F32, tag="hp")
            nc.tensor.matmul(hp, lhsT=ones_w, rhs=bp_t, start=True, stop=False)
            for k in range(KT):
                nc.tensor.matmul(
                    hp,
                    lhsT=xt[:, k * P:(k + 1) * P],
                    rhs=wp_t[k],
                    start=False,
                    stop=(k == KT - 1),
                )

            hout = hstore[:, g * H:(g + 1) * H]
            nc.scalar.activation(out=hout, in_=hp, func=act)

            scr = scrp.tile([P, H], BF, tag="scr")
            # split the elementwise mul between GpSimd and DVE; the final
            # chunks go to DVE (faster) to shorten the pass-2 gating chain
            mul_eng = nc.vector if g % 2 == 1 or g + 4 >= G else nc.gpsimd
            mul_eng.tensor_mul(scr, hout, wab_t[:, 0:H])
            nc.vector.reduce_sum(
                out=a_all[:, g:g + 1], in_=scr, axis=mybir.AxisListType.X
            )

        # ---- pass 2: one global masked exp (avoids ACT table-set thrash
        # between Silu and Exp), then per-bag pooled accumulation ----
        am = store.tile([P, G], F32, tag="am")
        nc.vector.tensor_add(am, a_all, mask_t)
        nc.scalar.activation(
            out=w_all, in_=am, func=mybir.ActivationFunctionType.Exp
        )
        off = 0
        for b in range(B):
            nb = n_per_bag[b]
            pool_t = poolp.tile([1, H], F32, tag="pool")
            for j in range(nb):
                g = off + j
                nc.tensor.matmul(
                    pool_t,
                    lhsT=w_all[:, g:g + 1],
                    rhs=hstore[:, g * H:(g + 1) * H],
                    start=(j == 0),
                    stop=(j == nb - 1),
                )
            nc.vector.tensor_copy(out_sb[0:1, b * H:(b + 1) * H], pool_t)
            nc.sync.dma_start(
                out=out[0:1, b * H:(b + 1) * H],
                in_=out_sb[0:1, b * H:(b + 1) * H],
            )
            off += nb

        s_ps = sps.tile([G, 1], F32, tag="sps")
        nc.tensor.matmul(s_ps, lhsT=w_all, rhs=ones_n, start=True, stop=True)
        s_sb = smallp.tile([G, 1], F32, tag="ssb")
        nc.vector.tensor_copy(s_sb, s_ps)
        nc.sync.dma_start(out=out[0:1, B * H:B * H + G], in_=s_sb)

    nc.compile()
    return nc


def _plan(lengths: np.ndarray):
    """Chunk counts: bag b has T_b valid chunks; every core gets n_b slots."""
    lens = np.asarray(lengths, dtype=np.int64)
    T = np.maximum((lens + P - 1) // P, 1)       # valid chunks per bag
    n = (T + NCORES - 1) // NCORES               # per-core slots per bag
    G = int(n.sum())
    return T, n, G


def _pack(x, lengths, T, n, G):
    """Per-core inputs: xpk [G,128,1024] bf16 (lhsT layout) + maskT [128,G]."""
    lens = np.asarray(lengths, dtype=np.int64)
    # x[b, t*128+p, k*128+d] -> xr[b, t, d, k*128+p]  (d = within-k-tile index)
    xr = (
        np.asarray(x)
        .astype(ml_dtypes.bfloat16)
        .reshape(B, N // P, P, KT, P)
        .transpose(0, 1, 4, 3, 2)
        .reshape(B, N // P, P, D)
    )
    bs = np.repeat(np.arange(B), n)
    in_maps = []
    for c in range(NCORES):
        js = np.concatenate([np.arange(nb) for nb in n])
        ts = c + NCORES * js                       # global chunk id per slot
        ts_clip = np.minimum(ts, T[bs] - 1)
        xpk = xr[bs, ts_clip]                      # [G, 128, 1024] bf16
        # partition-major: one contiguous G*2KB run per partition
        xpk_t = np.ascontiguousarray(xpk.transpose(1, 0, 2)).reshape(P, G * D)
        # valid patches in slot: clip(len - t*128, 0, 128); dummies get 0
        valid = np.clip(lens[bs] - ts * P, 0, P)
        valid[ts >= T[bs]] = 0
        maskT = np.where(
            np.arange(P)[:, None] < valid[None, :], 0.0, NEG
        ).astype(np.float32)
        in_maps.append({"xpk": xpk_t, "maskT": maskT})
    return in_maps, bs


def _run(inputs: dict, trace: bool = False):
    x = np.asarray(inputs["x"], dtype=np.float32)
    lengths = np.asarray(inputs["lengths"])
    Wp = np.asarray(inputs["Wp"], dtype=np.float32)
    bp = np.asarray(inputs["bp"], dtype=np.float32)
    Wa = np.asarray(inputs["Wa"], dtype=np.float32)
    Wc = np.asarray(inputs["Wc"], dtype=np.float32)
    bc = np.asarray(inputs["bc"], dtype=np.float32)

    T, n, G = _plan(lengths)
    key = (G, tuple(int(v) for v in n))
    if key not in _cache:
        _cache[key] = _build(G, key[1])
    nc = _cache[key]

    in_maps, bs = _pack(x, lengths, T, n, G)
    CW = KT * H + 4 * H + H
    cblob = np.zeros((P, CW), dtype=ml_dtypes.bfloat16)
    # wp k-tiles: cblob[d_in, k*H + h] = Wp[k*P + d_in, h]
    cblob[:, 0:KT * H] = (
        Wp.reshape(KT, P, H).transpose(1, 0, 2).reshape(P, KT * H)
        .astype(ml_dtypes.bfloat16)
    )
    cblob[:, KT * H:KT * H + 4 * H] = np.tile(
        Wa[:, 0][None, :], (P, 4)
    ).astype(ml_dtypes.bfloat16)
    cblob[0, KT * H + 4 * H:CW] = bp.astype(ml_dtypes.bfloat16)
    for m in in_maps:
        m["cblob"] = cblob

    res = run_bass_kernel_spmd(
        nc, in_maps, core_ids=list(range(NCORES)), trace=trace
    )

    v = np.zeros((B, H), np.float64)
    s = np.zeros(B, np.float64)
    for r in res.results:
        flat = r["out"].reshape(-1).astype(np.float64)
        v += flat[:B * H].reshape(B, H)
        np.add.at(s, bs, flat[B * H:B * H + G])
    pooled = v / s[:, None]
    logits = pooled @ Wc.astype(np.float64) + bc.astype(np.float64)
    return logits.astype(np.float32), res.exec_time_ns


def kernel(**inputs) -> np.ndarray:
    logits, _ = _run(inputs, trace=False)
    return logits
